# revision 53
# baseline (speedup 1.0000x reference)
"""Trainium2 Bass kernel for nn_BigramBaseline: causal mean pooling over
embedding-gathered rows.

  logits[b*T + t, :] = mean_{s<=t} emb[idx[b, s], :]

Strategy (data-parallel over batch, one batch row per core):
  - emb is cast to bf16 on host; the device gathers bf16 rows. Outputs are
    written as int8 with a host-computed per-token scale (the host knows
    idx, so it computes the exact per-token sigma of the prefix mean from
    row multiplicities) and dequantized to f32 on host. Combined rounding
    error ~1.4% rel, under the 2e-2 gate; HBM traffic drops 64 -> ~25 MiB
    per core.
  - blocks of 127 tokens; the running carry rides as ROW 0 of each block's
    rhs tile: the prefix mask's first row is all-ones, so one matmul per
    chunk computes carry + in-block prefix directly (no second "strict"
    matmul, no PSUM carry residency -- banks recycle with start=True and
    the cross-block chain is just a cheap single-partition copy of the
    block's last unscaled row into the next block's x-tile row 0).
  - per block: two half-row indirect gathers -> x[1:128, :]; 8 matmuls;
    scale-copies to int8 split scalar/vector engine by column half, with
    the carry-row copies interleaved so the next block's matmuls unblock
    chunk by chunk.
"""

import numpy as np
import ml_dtypes

B, T, V = 8, 2048, 4096
P = 128
L = P - 1          # tokens per block; row 0 of the rhs carries the prefix sum
CHUNK = 512
N_CORES = 8

INT8_OUT = True
# Headroom multiple of the exact per-token sigma: covers the max of 4096
# gaussian samples (~3.7 sigma); rel quant err = 4.6/(127*sqrt(12)) ~ 1%.
Q_SIGMA = 4.6


def block_spans(t=T):
    """[(t0, length)] covering T in blocks of up to L tokens."""
    spans = []
    t0 = 0
    while t0 < t:
        spans.append((t0, min(L, t - t0)))
        t0 += L
    return spans


def build_bass(t=T, v=V, int8_out=INT8_OUT):
    import concourse.bacc as bacc
    import concourse.bass as bass
    import concourse.tile as tile
    from concourse import mybir

    spans = block_spans(t)
    nblk = len(spans)
    chunk = min(CHUNK, v)
    nchunk = v // chunk

    mm_dt = mybir.dt.bfloat16
    out_dt = mybir.dt.int8 if int8_out else mm_dt

    nc = bacc.Bacc(trn_type="TRN2")
    emb = nc.declare_dram_parameter("emb", [v, v], mm_dt, isOutput=False)
    idx = nc.declare_dram_parameter("idx", [P, nblk], mybir.dt.int32, isOutput=False)
    invd = nc.declare_dram_parameter("invd", [P, nblk], mybir.dt.float32, isOutput=False)
    # Column 0 is the carry+colsum output (all-ones): acc partition 0 holds
    # carry_{k+1} so the carry-row copy is a same-partition op. Column 1+p is
    # token p's output: carry row plus the in-block causal prefix.
    masks = nc.declare_dram_parameter("masks", [P, P], mm_dt, isOutput=False)
    out = nc.declare_dram_parameter("out", [t, v], out_dt, isOutput=True)

    with tile.TileContext(nc) as tc:
        with (
            tc.tile_pool(name="const", bufs=1) as cpool,
            tc.tile_pool(name="x", bufs=4) as xpool,
            tc.tile_pool(name="o", bufs=4) as opool,
            tc.tile_pool(name="acc", bufs=1, space="PSUM") as ppool,
        ):
            idx_sb = cpool.tile([P, nblk], mybir.dt.int32)
            nc.sync.dma_start(out=idx_sb[:], in_=idx[:])
            invd_sb = cpool.tile([P, nblk], mybir.dt.float32)
            nc.sync.dma_start(out=invd_sb[:], in_=invd[:])
            masks_sb = cpool.tile([P, P], mm_dt)
            nc.sync.dma_start(out=masks_sb[:], in_=masks[:])

            acc = [
                ppool.tile([P, chunk], mybir.dt.float32, name=f"acc{c}", tag=f"acc{c}")
                for c in range(nchunk)
            ]

            # Warm-up ops: absorb each engine's constant-DMA wait (walrus fits
            # one sync wait per instruction) and trip the PE activity monitor
            # to full clock during the otherwise-dead startup window.
            for w in range(16):
                nc.tensor.matmul(
                    out=acc[0][:, 0:128],
                    lhsT=masks_sb[:],
                    rhs=masks_sb[:],
                    start=True,
                    stop=True,
                    skip_group_check=True,
                )
            scratch = cpool.tile([P, 1], mybir.dt.float32)
            nc.scalar.activation(
                out=scratch[:],
                in_=invd_sb[:, 0:1],
                func=mybir.ActivationFunctionType.Copy,
            )
            scratch2 = cpool.tile([P, 1], mybir.dt.float32)
            nc.vector.tensor_scalar_mul(scratch2[:], invd_sb[:, 0:1], invd_sb[:, 0:1])

            half = v // 2
            hchunk = nchunk // 2
            xt = [None] * nblk
            for k, (t0, ln) in enumerate(spans):
                if xt[k] is None:
                    xt[k] = xpool.tile([P, v], mm_dt, name="x")
                x = xt[k]
                if k == 0:
                    # Block 0 has no carry: zero row 0 before the matmuls.
                    nc.gpsimd.memset(x[0:1, :], 0.0)
                for h in range(2):
                    nc.gpsimd.indirect_dma_start(
                        out=x[1 : 1 + ln, h * half : (h + 1) * half],
                        out_offset=None,
                        in_=emb[:],
                        in_offset=bass.IndirectOffsetOnAxis(
                            ap=idx_sb[0:ln, k : k + 1], axis=0
                        ),
                        element_offset=h * half,
                    )
                o = opool.tile([P, v], out_dt)
                for c in range(nchunk):
                    nc.tensor.matmul(
                        out=acc[c][0 : 1 + ln],
                        lhsT=masks_sb[0 : 1 + ln, 0 : 1 + ln],
                        rhs=x[0 : 1 + ln, bass.ts(c, chunk)],
                        start=True,
                        stop=True,
                        skip_group_check=True,
                    )
                # Prepare the next block's carry row before the bulk copies:
                # carry_{k+1} = last token's unscaled row, written straight
                # into x_{k+1}[0:1] chunk by chunk so the next block's
                # matmuls unblock early. Interleave with the scale-copies.
                nxt = None
                if k + 1 < nblk:
                    xt[k + 1] = xpool.tile([P, v], mm_dt, name="x")
                    nxt = xt[k + 1]
                cp = P if ln == L else 32  # partition-0-aligned copy range
                for c in range(nchunk):
                    sl = bass.ts(c, chunk)
                    eng_act = c < hchunk
                    if nxt is not None:
                        if eng_act:
                            nc.scalar.activation(
                                out=nxt[0:1, sl],
                                in_=acc[c][0:1],
                                func=mybir.ActivationFunctionType.Copy,
                            )
                        else:
                            nc.vector.tensor_scalar_mul(
                                nxt[0:1, sl], acc[c][0:1], 1.0
                            )
                    if eng_act:
                        nc.scalar.activation(
                            out=o[0:cp, sl],
                            in_=acc[c][0:cp],
                            func=mybir.ActivationFunctionType.Copy,
                            scale=invd_sb[0:cp, k : k + 1],
                        )
                    else:
                        nc.vector.tensor_scalar_mul(
                            o[0:cp, sl], acc[c][0:cp], invd_sb[0:cp, k : k + 1]
                        )
                for h in range(2):
                    csl = slice(h * half, (h + 1) * half)
                    nc.sync.dma_start(
                        out=out[t0 : t0 + ln, csl], in_=o[1 : 1 + ln, csl]
                    )
                # Dead write into the just-shipped tile: routes the output
                # DMA's completion through the half's writer engine, so the
                # pool-slot reuse a few blocks later costs the next
                # scale-copy no extra sync wait.
                nc.scalar.activation(
                    out=o[:, 0:1],
                    in_=invd_sb[:, 0:1],
                    func=mybir.ActivationFunctionType.Copy,
                )
                nc.vector.tensor_scalar_mul(
                    o[:, half : half + 1], invd_sb[:, 0:1], invd_sb[:, 0:1]
                )
    nc.finalize()
    return nc


def host_inputs(idx_row, emb_bf16, t=T, v=V, int8_out=INT8_OUT):
    """Per-core input map for one batch row. Returns (in_map, deq or None)."""
    spans = block_spans(t)
    nblk = len(spans)
    idx_row = np.asarray(idx_row, dtype=np.int64)

    inv_t = 1.0 / np.arange(1, t + 1, dtype=np.float64)
    if int8_out:
        # Exact per-token sigma of the prefix mean: sqrt(sum of squared
        # multiplicities of the gathered rows over the causal prefix)/(t+1).
        counts = np.zeros(v, dtype=np.int64)
        sumsq = np.empty(t, dtype=np.float64)
        run = 0
        for s, r in enumerate(idx_row):
            run += 2 * counts[r] + 1
            counts[r] += 1
            sumsq[s] = run
        s_t = Q_SIGMA * np.sqrt(sumsq) * inv_t / 127.0
        q = inv_t / s_t          # device: int8 code = PSUM * q
        deq = s_t.astype(np.float32)
    else:
        q = inv_t
        deq = None

    # [P, nblk] layouts: token t0+p of block k -> [p, k]; unused cells inert.
    idx32 = np.zeros((P, nblk), dtype=np.int32)
    invd = np.ones((P, nblk), dtype=np.float32)
    for k, (t0, ln) in enumerate(spans):
        idx32[0:ln, k] = idx_row[t0 : t0 + ln]
        invd[1 : 1 + ln, k] = q[t0 : t0 + ln]

    maskC = np.zeros((P, P), dtype=ml_dtypes.bfloat16)
    maskC[:, 0] = 1          # acc partition 0 = carry + column sum
    maskC[0, :] = 1          # carry row feeds every output
    maskC[1:, 1:] = np.triu(np.ones((P - 1, P - 1), dtype=ml_dtypes.bfloat16))
    return {
        "emb": emb_bf16,
        "idx": np.ascontiguousarray(idx32),
        "invd": np.ascontiguousarray(invd),
        "masks": np.ascontiguousarray(maskC),
    }, deq


_nc_cache = {}


def kernel(idx, emb, _trace=False):
    from concourse.bass_utils import run_bass_kernel_spmd

    key = "nc"
    if key not in _nc_cache:
        _nc_cache[key] = build_bass()
    nc = _nc_cache[key]

    idx = np.asarray(idx)
    emb_bf16 = np.ascontiguousarray(np.asarray(emb).astype(ml_dtypes.bfloat16))
    in_maps = []
    deqs = []
    for b in range(N_CORES):
        m, deq = host_inputs(idx[b], emb_bf16)
        in_maps.append(m)
        deqs.append(deq)
    res = run_bass_kernel_spmd(nc, in_maps, list(range(N_CORES)), trace=_trace)
    kernel.last_results = res
    outs = []
    for b, r in enumerate(res.results):
        o = np.asarray(r["out"])
        if INT8_OUT:
            o = o.astype(np.float32) * deqs[b][:, None]
        else:
            o = o.astype(np.float32)
        outs.append(o)
    return np.concatenate(outs, axis=0)


# revision 54
# speedup vs baseline: 1.0006x; 1.0006x over previous
"""Trainium2 Bass kernel for nn_BigramBaseline: causal mean pooling over
embedding-gathered rows.

  logits[b*T + t, :] = mean_{s<=t} emb[idx[b, s], :]

Strategy (data-parallel over batch, one batch row per core):
  - emb is cast to bf16 on host; the device gathers bf16 rows. Outputs are
    written as int8 with a host-computed per-token scale (the host knows
    idx, so it computes the exact per-token sigma of the prefix mean from
    row multiplicities) and dequantized to f32 on host. Combined rounding
    error ~1.4% rel, under the 2e-2 gate; HBM traffic drops 64 -> ~25 MiB
    per core.
  - blocks of 127 tokens; the running carry rides as ROW 0 of each block's
    rhs tile: the prefix mask's first row is all-ones, so one matmul per
    chunk computes carry + in-block prefix directly (no second "strict"
    matmul, no PSUM carry residency -- banks recycle with start=True and
    the cross-block chain is just a cheap single-partition copy of the
    block's last unscaled row into the next block's x-tile row 0).
  - per block: two half-row indirect gathers -> x[1:128, :]; 8 matmuls;
    scale-copies to int8 split scalar/vector engine by column half, with
    the carry-row copies interleaved so the next block's matmuls unblock
    chunk by chunk.
"""

import numpy as np
import ml_dtypes

B, T, V = 8, 2048, 4096
P = 128
L = P - 1          # tokens per block; row 0 of the rhs carries the prefix sum
CHUNK = 512
N_CORES = 8

INT8_OUT = True
# Headroom multiple of the exact per-token sigma: covers the max of 4096
# gaussian samples (~3.7 sigma); rel quant err = 4.6/(127*sqrt(12)) ~ 1%.
Q_SIGMA = 4.6


def block_spans(t=T):
    """[(t0, length)] covering T in blocks of up to L tokens."""
    spans = []
    t0 = 0
    while t0 < t:
        spans.append((t0, min(L, t - t0)))
        t0 += L
    return spans


def build_bass(t=T, v=V, int8_out=INT8_OUT):
    import concourse.bacc as bacc
    import concourse.bass as bass
    import concourse.tile as tile
    from concourse import mybir

    spans = block_spans(t)
    nblk = len(spans)
    chunk = min(CHUNK, v)
    nchunk = v // chunk

    mm_dt = mybir.dt.bfloat16
    out_dt = mybir.dt.int8 if int8_out else mm_dt

    nc = bacc.Bacc(trn_type="TRN2")
    emb = nc.declare_dram_parameter("emb", [v, v], mm_dt, isOutput=False)
    idx = nc.declare_dram_parameter("idx", [P, nblk], mybir.dt.int32, isOutput=False)
    invd = nc.declare_dram_parameter("invd", [P, nblk], mybir.dt.float32, isOutput=False)
    # M2[s, 0] = 1 for all s (acc partition 0 = column sum, and with the
    # carry inject it becomes carry_{k+1}); M2[s, 1+p] = 1 iff s <= p (token
    # p's causal prefix at partition 1+p). Row 0 is all ones and doubles as
    # the lhsT of the K=1 carry-inject matmul.
    masks = nc.declare_dram_parameter("masks", [P, P], mm_dt, isOutput=False)
    out = nc.declare_dram_parameter("out", [t, v], out_dt, isOutput=True)

    with tile.TileContext(nc) as tc:
        with (
            tc.tile_pool(name="const", bufs=1) as cpool,
            tc.tile_pool(name="x", bufs=4) as xpool,
            tc.tile_pool(name="o", bufs=4) as opool,
            tc.tile_pool(name="cs", bufs=4) as cspool,
            tc.tile_pool(name="acc", bufs=1, space="PSUM") as ppool,
        ):
            idx_sb = cpool.tile([P, nblk], mybir.dt.int32)
            nc.sync.dma_start(out=idx_sb[:], in_=idx[:])
            invd_sb = cpool.tile([P, nblk], mybir.dt.float32)
            nc.sync.dma_start(out=invd_sb[:], in_=invd[:])
            masks_sb = cpool.tile([P, P], mm_dt)
            nc.sync.dma_start(out=masks_sb[:], in_=masks[:])

            acc = [
                ppool.tile([P, chunk], mybir.dt.float32, name=f"acc{c}", tag=f"acc{c}")
                for c in range(nchunk)
            ]

            # Warm-up ops: absorb each engine's constant-DMA wait (walrus fits
            # one sync wait per instruction) and trip the PE activity monitor
            # to full clock during the otherwise-dead startup window.
            for w in range(16):
                nc.tensor.matmul(
                    out=acc[0][:, 0:128],
                    lhsT=masks_sb[:],
                    rhs=masks_sb[:],
                    start=True,
                    stop=True,
                    skip_group_check=True,
                )
            scratch = cpool.tile([P, 1], mybir.dt.float32)
            nc.scalar.activation(
                out=scratch[:],
                in_=invd_sb[:, 0:1],
                func=mybir.ActivationFunctionType.Copy,
            )
            scratch2 = cpool.tile([P, 1], mybir.dt.float32)
            nc.vector.tensor_scalar_mul(scratch2[:], invd_sb[:, 0:1], invd_sb[:, 0:1])

            half = v // 2
            hchunk = nchunk // 2
            xt = [None] * nblk
            cs = [None] * nblk
            for k, (t0, ln) in enumerate(spans):
                if xt[k] is None:
                    xt[k] = xpool.tile([P, v], mm_dt, name="x")
                x = xt[k]
                if k == 0:
                    cs[0] = cspool.tile([1, v], mm_dt, name="cs")
                    nc.gpsimd.memset(cs[0][0:1, :], 0.0)
                for h in range(2):
                    nc.gpsimd.indirect_dma_start(
                        out=x[0:ln, h * half : (h + 1) * half],
                        out_offset=None,
                        in_=emb[:],
                        in_offset=bass.IndirectOffsetOnAxis(
                            ap=idx_sb[0:ln, k : k + 1], axis=0
                        ),
                        element_offset=h * half,
                    )
                o = opool.tile([P, v], out_dt)
                for c in range(nchunk):
                    nc.tensor.matmul(
                        out=acc[c][0 : 1 + ln],
                        lhsT=masks_sb[0:ln, 0 : 1 + ln],
                        rhs=x[0:ln, bass.ts(c, chunk)],
                        start=True,
                        stop=False,
                        skip_group_check=True,
                    )
                for c in range(nchunk):
                    # K=1 carry inject: adds carry_k to every output partition.
                    nc.tensor.matmul(
                        out=acc[c][0 : 1 + ln],
                        lhsT=masks_sb[0:1, 0 : 1 + ln],
                        rhs=cs[k][0:1, bass.ts(c, chunk)],
                        start=False,
                        stop=True,
                        skip_group_check=True,
                    )
                # Prepare the next block's carry row before the bulk copies:
                # carry_{k+1} = last token's unscaled row, written straight
                # into x_{k+1}[0:1] chunk by chunk so the next block's
                # matmuls unblock early. Interleave with the scale-copies.
                nxt = None
                if k + 1 < nblk:
                    xt[k + 1] = xpool.tile([P, v], mm_dt, name="x")
                    cs[k + 1] = cspool.tile([1, v], mm_dt, name="cs")
                    nxt = cs[k + 1]
                cp = P if ln == L else 32  # partition-0-aligned copy range
                for c in range(nchunk):
                    sl = bass.ts(c, chunk)
                    eng_act = c < hchunk
                    if nxt is not None:
                        if eng_act:
                            nc.scalar.activation(
                                out=nxt[0:1, sl],
                                in_=acc[c][0:1],
                                func=mybir.ActivationFunctionType.Copy,
                            )
                        else:
                            nc.vector.tensor_scalar_mul(
                                nxt[0:1, sl], acc[c][0:1], 1.0
                            )
                    if eng_act:
                        nc.scalar.activation(
                            out=o[0:cp, sl],
                            in_=acc[c][0:cp],
                            func=mybir.ActivationFunctionType.Copy,
                            scale=invd_sb[0:cp, k : k + 1],
                        )
                    else:
                        nc.vector.tensor_scalar_mul(
                            o[0:cp, sl], acc[c][0:cp], invd_sb[0:cp, k : k + 1]
                        )
                for h in range(2):
                    csl = slice(h * half, (h + 1) * half)
                    nc.sync.dma_start(
                        out=out[t0 : t0 + ln, csl], in_=o[1 : 1 + ln, csl]
                    )
                # Dead write into the just-shipped tile: routes the output
                # DMA's completion through the half's writer engine, so the
                # pool-slot reuse a few blocks later costs the next
                # scale-copy no extra sync wait.
                nc.scalar.activation(
                    out=o[:, 0:1],
                    in_=invd_sb[:, 0:1],
                    func=mybir.ActivationFunctionType.Copy,
                )
                nc.vector.tensor_scalar_mul(
                    o[:, half : half + 1], invd_sb[:, 0:1], invd_sb[:, 0:1]
                )
    nc.finalize()
    return nc


def host_inputs(idx_row, emb_bf16, t=T, v=V, int8_out=INT8_OUT):
    """Per-core input map for one batch row. Returns (in_map, deq or None)."""
    spans = block_spans(t)
    nblk = len(spans)
    idx_row = np.asarray(idx_row, dtype=np.int64)

    inv_t = 1.0 / np.arange(1, t + 1, dtype=np.float64)
    if int8_out:
        # Exact per-token sigma of the prefix mean: sqrt(sum of squared
        # multiplicities of the gathered rows over the causal prefix)/(t+1).
        counts = np.zeros(v, dtype=np.int64)
        sumsq = np.empty(t, dtype=np.float64)
        run = 0
        for s, r in enumerate(idx_row):
            run += 2 * counts[r] + 1
            counts[r] += 1
            sumsq[s] = run
        s_t = Q_SIGMA * np.sqrt(sumsq) * inv_t / 127.0
        q = inv_t / s_t          # device: int8 code = PSUM * q
        deq = s_t.astype(np.float32)
    else:
        q = inv_t
        deq = None

    # [P, nblk] layouts: token t0+p of block k -> [p, k]; unused cells inert.
    idx32 = np.zeros((P, nblk), dtype=np.int32)
    invd = np.ones((P, nblk), dtype=np.float32)
    for k, (t0, ln) in enumerate(spans):
        idx32[0:ln, k] = idx_row[t0 : t0 + ln]
        invd[1 : 1 + ln, k] = q[t0 : t0 + ln]

    maskC = np.zeros((P, P), dtype=ml_dtypes.bfloat16)
    maskC[:, 0] = 1          # acc partition 0 = column sum (+ carry inject)
    maskC[0:P - 1, 1:] = np.triu(np.ones((P - 1, P - 1), dtype=ml_dtypes.bfloat16))
    return {
        "emb": emb_bf16,
        "idx": np.ascontiguousarray(idx32),
        "invd": np.ascontiguousarray(invd),
        "masks": np.ascontiguousarray(maskC),
    }, deq


_nc_cache = {}


def kernel(idx, emb, _trace=False):
    from concourse.bass_utils import run_bass_kernel_spmd

    key = "nc"
    if key not in _nc_cache:
        _nc_cache[key] = build_bass()
    nc = _nc_cache[key]

    idx = np.asarray(idx)
    emb_bf16 = np.ascontiguousarray(np.asarray(emb).astype(ml_dtypes.bfloat16))
    in_maps = []
    deqs = []
    for b in range(N_CORES):
        m, deq = host_inputs(idx[b], emb_bf16)
        in_maps.append(m)
        deqs.append(deq)
    res = run_bass_kernel_spmd(nc, in_maps, list(range(N_CORES)), trace=_trace)
    kernel.last_results = res
    outs = []
    for b, r in enumerate(res.results):
        o = np.asarray(r["out"])
        if INT8_OUT:
            o = o.astype(np.float32) * deqs[b][:, None]
        else:
            o = o.astype(np.float32)
        outs.append(o)
    return np.concatenate(outs, axis=0)


# revision 55
# speedup vs baseline: 3.7084x; 3.7063x over previous
"""Trainium2 Bass kernel for nn_BigramBaseline: causal mean pooling over
embedding-gathered rows.  (v2 snapshot: measured 112158 ns, rel err 2.29e-3)

  logits[b*T + t, :] = mean_{s<=t} emb[idx[b, s], :]

Strategy (data-parallel over batch, one batch row per core):
  - emb is cast to bf16 on host; the device gathers bf16 rows and writes
    bf16 outputs (upcast to f32 on host). Rounding error ~0.3% rel, well
    under the 2e-2 gate, and it halves HBM traffic both ways: 64 MiB ->
    32 MiB per core, which is what matters in this memory-bound regime.
  - per 128-token block: indirect-DMA gather of 128 emb rows -> SBUF
    tile [128, V] (partition = token within block)
  - in-block causal prefix sum via PE matmul with a lower-triangular
    ones matrix (lhsT = upper-triangular incl. diag)
  - cross-block carry kept resident in PSUM: after emitting the block's
    prefix sums, a second matmul with the strict complement mask adds
    the rest of the block's column-sums, turning the PSUM bank into
    carry_{k+1} broadcast over all 128 partitions
  - scale by 1/(t+1) during the PSUM->SBUF copy (per-partition scale
    operand); chunks 0-3 go through the scalar engine, 4-7 through the
    vector engine, splitting the copy load
  - tril/strict matmuls are batched per block (all 8 tril, then all 8
    strict) so the PE swaps weights twice per block instead of 16 times
"""

import numpy as np
import ml_dtypes

B, T, V = 8, 2048, 4096
P = 128
CHUNK = 512
N_CORES = 8


def build_bass(t=T, v=V):
    import concourse.bacc as bacc
    import concourse.bass as bass
    import concourse.tile as tile
    from concourse import mybir

    nblk = t // P
    chunk = min(CHUNK, v)
    nchunk = v // chunk

    mm_dt = mybir.dt.bfloat16

    nc = bacc.Bacc(trn_type="TRN2")
    emb = nc.declare_dram_parameter("emb", [v, v], mm_dt, isOutput=False)
    idx = nc.declare_dram_parameter("idx", [P, nblk], mybir.dt.int32, isOutput=False)
    invd = nc.declare_dram_parameter("invd", [P, nblk], mybir.dt.float32, isOutput=False)
    masks = nc.declare_dram_parameter("masks", [P, 2 * P], mm_dt, isOutput=False)
    out = nc.declare_dram_parameter("out", [t, v], mm_dt, isOutput=True)

    with tile.TileContext(nc) as tc:
        with (
            tc.tile_pool(name="const", bufs=1) as cpool,
            tc.tile_pool(name="x", bufs=4) as xpool,
            tc.tile_pool(name="o", bufs=4) as opool,
            tc.tile_pool(name="acc", bufs=1, space="PSUM") as ppool,
        ):
            idx_sb = cpool.tile([P, nblk], mybir.dt.int32)
            nc.sync.dma_start(out=idx_sb[:], in_=idx[:])
            invd_sb = cpool.tile([P, nblk], mybir.dt.float32)
            nc.sync.dma_start(out=invd_sb[:], in_=invd[:])
            masks_sb = cpool.tile([P, 2 * P], mm_dt)
            nc.sync.dma_start(out=masks_sb[:], in_=masks[:])
            trilT_sb = masks_sb[:, 0:P]
            strictT_sb = masks_sb[:, P : 2 * P]

            acc = [
                ppool.tile([P, chunk], mybir.dt.float32, name=f"acc{c}", tag=f"acc{c}")
                for c in range(nchunk)
            ]

            for w in range(16):
                nc.tensor.matmul(
                    out=acc[0][:, 0:256],
                    lhsT=trilT_sb,
                    rhs=masks_sb[:, 0:256],
                    start=True,
                    stop=True,
                    skip_group_check=True,
                )
            scratch = cpool.tile([P, 1], mybir.dt.float32)
            nc.scalar.activation(
                out=scratch[:],
                in_=invd_sb[:, 0:1],
                func=mybir.ActivationFunctionType.Copy,
            )
            scratch2 = cpool.tile([P, 1], mybir.dt.float32)
            nc.vector.tensor_scalar_mul(scratch2[:], invd_sb[:, 0:1], invd_sb[:, 0:1])

            half = v // 2
            hchunk = nchunk // 2
            for k in range(nblk):
                x = xpool.tile([P, v], mm_dt)
                for h in range(2):
                    nc.gpsimd.indirect_dma_start(
                        out=x[:, h * half : (h + 1) * half],
                        out_offset=None,
                        in_=emb[:],
                        in_offset=bass.IndirectOffsetOnAxis(
                            ap=idx_sb[:, k : k + 1], axis=0
                        ),
                        element_offset=h * half,
                    )
                o = opool.tile([P, v], mm_dt)
                for c in range(nchunk):
                    nc.tensor.matmul(
                        out=acc[c][:],
                        lhsT=trilT_sb,
                        rhs=x[:, bass.ts(c, chunk)],
                        start=(k == 0),
                        stop=True,
                        skip_group_check=True,
                    )
                for c in range(nchunk):
                    sl = bass.ts(c, chunk)
                    if c < hchunk:
                        nc.scalar.activation(
                            out=o[:, sl],
                            in_=acc[c][:],
                            func=mybir.ActivationFunctionType.Copy,
                            scale=invd_sb[:, k : k + 1],
                        )
                    else:
                        nc.vector.tensor_scalar_mul(
                            o[:, sl], acc[c][:], invd_sb[:, k : k + 1]
                        )
                if k < nblk - 1:
                    for c in range(nchunk):
                        nc.tensor.matmul(
                            out=acc[c][:],
                            lhsT=strictT_sb,
                            rhs=x[:, bass.ts(c, chunk)],
                            start=False,
                            stop=True,
                            skip_group_check=True,
                        )
                for h in range(2):
                    csl = slice(h * half, (h + 1) * half)
                    nc.sync.dma_start(
                        out=out[bass.ts(k, P), csl], in_=o[:, csl]
                    )
                nc.scalar.activation(
                    out=o[:, 0:1],
                    in_=invd_sb[:, 0:1],
                    func=mybir.ActivationFunctionType.Copy,
                )
                nc.vector.tensor_scalar_mul(
                    o[:, half : half + 1], invd_sb[:, 0:1], invd_sb[:, 0:1]
                )
    nc.finalize()
    return nc


def host_inputs(idx_row, emb_bf16, t=T, v=V):
    nblk = t // P
    idx32 = np.ascontiguousarray(
        np.asarray(idx_row, dtype=np.int32).reshape(nblk, P).T
    )
    invd = np.ascontiguousarray(
        (1.0 / np.arange(1, t + 1, dtype=np.float64))
        .astype(np.float32)
        .reshape(nblk, P)
        .T
    )
    masks = np.concatenate(
        [
            np.triu(np.ones((P, P), dtype=ml_dtypes.bfloat16)),
            np.tril(np.ones((P, P), dtype=ml_dtypes.bfloat16), -1),
        ],
        axis=1,
    )
    return {
        "emb": emb_bf16,
        "idx": idx32,
        "invd": invd,
        "masks": np.ascontiguousarray(masks),
    }


_nc_cache = {}


def kernel(idx, emb, _trace=False):
    from concourse.bass_utils import run_bass_kernel_spmd

    key = "nc"
    if key not in _nc_cache:
        _nc_cache[key] = build_bass()
    nc = _nc_cache[key]

    idx = np.asarray(idx)
    emb_bf16 = np.ascontiguousarray(np.asarray(emb).astype(ml_dtypes.bfloat16))
    in_maps = [host_inputs(idx[b], emb_bf16) for b in range(N_CORES)]
    res = run_bass_kernel_spmd(nc, in_maps, list(range(N_CORES)), trace=_trace)
    kernel.last_results = res
    out = np.concatenate(
        [np.asarray(r["out"]).astype(np.float32) for r in res.results], axis=0
    )
    return out


# revision 57
# speedup vs baseline: 3.8735x; 1.0445x over previous
"""Trainium2 Bass kernel for nn_BigramBaseline: causal mean pooling over
embedding-gathered rows.  (v2 snapshot: measured 112158 ns, rel err 2.29e-3)

  logits[b*T + t, :] = mean_{s<=t} emb[idx[b, s], :]

Strategy (data-parallel over batch, one batch row per core):
  - emb is cast to bf16 on host; the device gathers bf16 rows and writes
    bf16 outputs (upcast to f32 on host). Rounding error ~0.3% rel, well
    under the 2e-2 gate, and it halves HBM traffic both ways: 64 MiB ->
    32 MiB per core, which is what matters in this memory-bound regime.
  - per 128-token block: indirect-DMA gather of 128 emb rows -> SBUF
    tile [128, V] (partition = token within block)
  - in-block causal prefix sum via PE matmul with a lower-triangular
    ones matrix (lhsT = upper-triangular incl. diag)
  - cross-block carry kept resident in PSUM: after emitting the block's
    prefix sums, a second matmul with the strict complement mask adds
    the rest of the block's column-sums, turning the PSUM bank into
    carry_{k+1} broadcast over all 128 partitions
  - scale by 1/(t+1) during the PSUM->SBUF copy (per-partition scale
    operand); chunks 0-3 go through the scalar engine, 4-7 through the
    vector engine, splitting the copy load
  - tril/strict matmuls are batched per block (all 8 tril, then all 8
    strict) so the PE swaps weights twice per block instead of 16 times
"""

import numpy as np
import ml_dtypes

B, T, V = 8, 2048, 4096
P = 128
CHUNK = 512
N_CORES = 8


def build_bass(t=T, v=V):
    import concourse.bacc as bacc
    import concourse.bass as bass
    import concourse.tile as tile
    from concourse import mybir

    nblk = t // P
    chunk = min(CHUNK, v)
    nchunk = v // chunk

    mm_dt = mybir.dt.bfloat16

    nc = bacc.Bacc(trn_type="TRN2")
    emb = nc.declare_dram_parameter("emb", [v, v], mm_dt, isOutput=False)
    idx = nc.declare_dram_parameter("idx", [P, nblk], mybir.dt.int32, isOutput=False)
    invd = nc.declare_dram_parameter("invd", [P, nblk], mybir.dt.float32, isOutput=False)
    masks = nc.declare_dram_parameter("masks", [P, 2 * P], mm_dt, isOutput=False)
    out = nc.declare_dram_parameter("out", [t, v], mm_dt, isOutput=True)

    with tile.TileContext(nc) as tc:
        with (
            tc.tile_pool(name="const", bufs=1) as cpool,
            tc.tile_pool(name="x", bufs=4) as xpool,
            tc.tile_pool(name="o", bufs=4) as opool,
            tc.tile_pool(name="acc", bufs=1, space="PSUM") as ppool,
        ):
            idx_sb = cpool.tile([P, nblk], mybir.dt.int32)
            nc.sync.dma_start(out=idx_sb[:], in_=idx[:])
            invd_sb = cpool.tile([P, nblk], mybir.dt.float32)
            nc.sync.dma_start(out=invd_sb[:], in_=invd[:])
            masks_sb = cpool.tile([P, 2 * P], mm_dt)
            nc.sync.dma_start(out=masks_sb[:], in_=masks[:])
            trilT_sb = masks_sb[:, 0:P]
            strictT_sb = masks_sb[:, P : 2 * P]

            acc = [
                ppool.tile([P, chunk], mybir.dt.float32, name=f"acc{c}", tag=f"acc{c}")
                for c in range(nchunk)
            ]

            for w in range(16):
                nc.tensor.matmul(
                    out=acc[0][:, 0:256],
                    lhsT=trilT_sb,
                    rhs=masks_sb[:, 0:256],
                    start=True,
                    stop=True,
                    skip_group_check=True,
                )
            scratch = cpool.tile([P, 1], mybir.dt.float32)
            nc.scalar.activation(
                out=scratch[:],
                in_=invd_sb[:, 0:1],
                func=mybir.ActivationFunctionType.Copy,
            )
            scratch2 = cpool.tile([P, 1], mybir.dt.float32)
            nc.vector.tensor_scalar_mul(scratch2[:], invd_sb[:, 0:1], invd_sb[:, 0:1])

            half = v // 2
            hchunk = nchunk // 2
            for k in range(nblk):
                x = xpool.tile([P, v], mm_dt)
                for h in range(2):
                    nc.gpsimd.indirect_dma_start(
                        out=x[:, h * half : (h + 1) * half],
                        out_offset=None,
                        in_=emb[:],
                        in_offset=bass.IndirectOffsetOnAxis(
                            ap=idx_sb[:, k : k + 1], axis=0
                        ),
                        element_offset=h * half,
                    )
                o = opool.tile([P, v], mm_dt)
                for c in range(nchunk):
                    nc.tensor.matmul(
                        out=acc[c][:],
                        lhsT=trilT_sb,
                        rhs=x[:, bass.ts(c, chunk)],
                        start=(k == 0),
                        stop=True,
                        skip_group_check=True,
                    )
                for c in range(nchunk):
                    sl = bass.ts(c, chunk)
                    if c < hchunk:
                        nc.scalar.activation(
                            out=o[:, sl],
                            in_=acc[c][:],
                            func=mybir.ActivationFunctionType.Copy,
                            scale=invd_sb[:, k : k + 1],
                        )
                    else:
                        nc.vector.tensor_scalar_mul(
                            o[:, sl], acc[c][:], invd_sb[:, k : k + 1]
                        )
                if k < nblk - 1:
                    for c in range(nchunk):
                        nc.tensor.matmul(
                            out=acc[c][:],
                            lhsT=strictT_sb,
                            rhs=x[:, bass.ts(c, chunk)],
                            start=False,
                            stop=True,
                            skip_group_check=True,
                        )
                for h in range(2):
                    csl = slice(h * half, (h + 1) * half)
                    nc.sync.dma_start(
                        out=out[bass.ts(k, P), csl], in_=o[:, csl]
                    )
                nc.scalar.activation(
                    out=o[:, 0:1],
                    in_=invd_sb[:, 0:1],
                    func=mybir.ActivationFunctionType.Copy,
                )
                nc.vector.tensor_scalar_mul(
                    o[:, half : half + 1], invd_sb[:, 0:1], invd_sb[:, 0:1]
                )
    nc.finalize()
    return nc


def host_inputs(idx_row, emb_bf16, t=T, v=V):
    nblk = t // P
    idx32 = np.ascontiguousarray(
        np.asarray(idx_row, dtype=np.int32).reshape(nblk, P).T
    )
    invd = np.ascontiguousarray(
        (1.0 / np.arange(1, t + 1, dtype=np.float64))
        .astype(np.float32)
        .reshape(nblk, P)
        .T
    )
    masks = np.concatenate(
        [
            np.triu(np.ones((P, P), dtype=ml_dtypes.bfloat16)),
            np.tril(np.ones((P, P), dtype=ml_dtypes.bfloat16), -1),
        ],
        axis=1,
    )
    return {
        "emb": emb_bf16,
        "idx": idx32,
        "invd": invd,
        "masks": np.ascontiguousarray(masks),
    }


_nc_cache = {}


def kernel(idx, emb, _trace=False):
    from concourse.bass_utils import run_bass_kernel_spmd

    key = "nc"
    if key not in _nc_cache:
        _nc_cache[key] = build_bass()
    nc = _nc_cache[key]

    idx = np.asarray(idx)
    emb_bf16 = np.ascontiguousarray(np.asarray(emb).astype(ml_dtypes.bfloat16))
    in_maps = [host_inputs(idx[b], emb_bf16) for b in range(N_CORES)]
    res = run_bass_kernel_spmd(nc, in_maps, list(range(N_CORES)), trace=_trace)
    kernel.last_results = res
    out = np.concatenate(
        [np.asarray(r["out"]).astype(np.float32) for r in res.results], axis=0
    )
    return out
